# revision 58
# baseline (speedup 1.0000x reference)
"""4-layer GCN (PyG GCNConv-style) on 8 Trainium2 NeuronCores.

Strategy (graph/data parallel, per sharding hint):
 - Nodes sharded by range across the 8 cores (6250 -> padded 6272 = 49*128 each).
 - Per layer: each core computes h = x_in @ W for its node shard (PE matmul),
   AllGathers h (bf16) so every core holds the full node-feature table, then
   gathers edge messages h[src] with the TIE-accelerated dma_gather and
   scatter-adds them into its own dst windows via PE matmuls against
   host-precomputed 128x128 selection matrices (norm folded in, fp8e4m3 and
   fully SBUF-resident across all 4 layers).
 - The gather is descriptor-rate bound, so each super-window's messages are
   split 4 ways across 4 SWDGE queues for parallel descriptor drain.
 - The node table is split into two window-chunks with independent AllGathers
   (double-buffered across layers): next-layer phase A is interleaved into
   phase C per-window, so chunk-0's AllGather overlaps the second half of the
   current layer's message passing. Each chunk's global row count fits int16,
   which also replaces the old lo/hi index-range split.
 - Self loops are folded in as one diag-matmul per 128-node window; bias via a
   K=1 matmul; ReLU / final log_softmax on ACT/DVE.

Numerics: bf16 storage/matmul operands (sel and the streamed x in fp8e4m3)
with f32 PSUM accumulation; ~3.6e-3 max relative error vs the f32 reference.
"""

import numpy as np
import ml_dtypes

import concourse.bass as bass
import concourse.tile as tile
from concourse import bacc, mybir
from concourse.bass_utils import run_bass_kernel_spmd

# problem constants (per spec nn_Net_33243046871554)
N_NODES = 50000
N_EDGES = 600000
D_IN = 2050
DH = 128
C = 8
NPC = N_NODES // C

BF16 = mybir.dt.bfloat16
F32 = mybir.dt.float32
F8 = mybir.dt.float8e4
I16 = mybir.dt.int16
BF = ml_dtypes.bfloat16
F8NP = ml_dtypes.float8_e4m3fn


def _cdiv(a, b):
    return -(-a // b)


class Cfg:
    def __init__(self, cores=8, n_nodes=N_NODES, d_in=D_IN, sww=4):
        assert n_nodes % cores == 0
        self.cores = cores
        self.n_nodes = n_nodes
        self.d_in = d_in
        self.npc = n_nodes // cores
        self.wpc = _cdiv(self.npc, 128)
        self.padn = self.wpc * 128
        self.kc = _cdiv(d_in, 128)
        self.kpad = self.kc * 128
        self.fullr = cores * self.padn
        self.sww = sww
        # window-chunk split for pipelined AllGathers; each chunk's global
        # row count must fit int16 (gather index dtype). Balanced split
        # measured best: chunk-0's AG fits its overlap window (second half
        # of the previous layer) and chunk-1's exposure is minimized given
        # the collective's 15us-fixed + bandwidth-ramp cost curve
        # (asymmetric 32/17 measured ~200us worse end-to-end).
        self.w0 = (self.wpc + 1) // 2
        self.wchunk = [self.w0, self.wpc - self.w0]
        for wk in self.wchunk:
            assert cores * wk * 128 <= 32768


def preprocess(cfg, x, edge_index, edge_attr, W1, b1, W2, b2, W3, b3, W4, b4,
               pad_idx=0):
    """Host-side graph preprocessing. Returns (in_maps, meta)."""
    x = np.asarray(x, np.float32)
    ei = np.asarray(edge_index)
    ea = np.asarray(edge_attr, np.float32)
    src = ei[0].astype(np.int64)
    dst = ei[1].astype(np.int64)
    Ws = [np.asarray(w, np.float32) for w in (W1, W2, W3, W4)]
    bs = [np.asarray(b, np.float32) for b in (b1, b2, b3, b4)]
    NC_, NPC_, WPC, KC = cfg.cores, cfg.npc, cfg.wpc, cfg.kc
    PADN, KPAD = cfg.padn, cfg.kpad
    NN = cfg.n_nodes
    W0 = cfg.w0

    deg = np.bincount(dst, weights=ea, minlength=NN) + 1.0
    dinv = (1.0 / np.sqrt(deg)).astype(np.float32)
    norm = (ea * dinv[src] * dinv[dst]).astype(np.float32)
    selfw = (dinv * dinv).astype(np.float32)

    core_e = dst // NPC_
    loc = dst - core_e * NPC_
    win_e = loc >> 7
    dl = loc & 127

    # source position -> (chunk, int16 row in that chunk's gathered table)
    s_core = src // NPC_
    s_loc = src - s_core * NPC_
    s_win = s_loc >> 7
    s_dl = s_loc & 127
    hi = (s_win >= W0).astype(np.int64)  # phase = source chunk
    rows_k = [W0 * 128, (WPC - W0) * 128]
    row16 = np.where(
        hi == 0,
        s_core * rows_k[0] + s_win * 128 + s_dl,
        s_core * rows_k[1] + (s_win - W0) * 128 + s_dl,
    )
    assert row16.max() < 32768
    idx16 = row16.astype(np.int16)

    # per (core, window, phase) edge counts -> shared padded block counts
    counts = np.zeros((NC_, WPC, 2), np.int64)
    np.add.at(counts, (core_e, win_e, hi), 1)
    blocks = _cdiv(counts, 128)
    Bmax = blocks.max(axis=0)          # [WPC, 2]
    BL = Bmax[:, 0].tolist()
    BH = Bmax[:, 1].tolist()

    # super-window grouping and global block layout:
    # for each sw: [chunk0 blocks of its windows][chunk1 blocks of its windows]
    sws_w = [list(range(s, min(s + cfg.sww, WPC))) for s in range(0, WPC, cfg.sww)]
    blk_base = np.zeros((WPC, 2), np.int64)
    sw_info = []
    tot = 0
    for sw in sws_w:
        lo0 = tot
        for w in sw:
            blk_base[w, 0] = tot
            tot += BL[w]
        hi0 = tot
        for w in sw:
            blk_base[w, 1] = tot
            tot += BH[w]
        sw_info.append(dict(windows=sw, lo_blk0=lo0, n_lo=hi0 - lo0,
                            hi_blk0=hi0, n_hi=tot - hi0))
    TOTBLK = tot
    TOTE = TOTBLK * 128

    meta = dict(BL=BL, BH=BH, sws=sw_info, blk_base=blk_base, TOTBLK=TOTBLK)

    # shared (replicated) weight layouts
    w1p = np.zeros((KPAD, DH), np.float32)
    w1p[:cfg.d_in] = Ws[0]
    W1H = np.ascontiguousarray(
        w1p.reshape(KC, 128, DH).transpose(1, 0, 2).reshape(128, KC * DH)
    ).astype(BF)
    W234 = [w.astype(BF) for w in Ws[1:]]
    BIAS = np.zeros((1, 4 * DH), np.float32)
    for i, b in enumerate(bs):
        BIAS[0, i * DH:(i + 1) * DH] = b
    BIAS = BIAS.astype(BF)
    ONES = np.ones((1, 128), BF)

    # per-core arrays
    in_maps = []
    eorder_key = core_e * (WPC * 2) + win_e * 2 + hi
    order = np.argsort(eorder_key, kind="stable")
    so_core, so_win, so_hi = core_e[order], win_e[order], hi[order]
    so_idx16, so_norm, so_dl = idx16[order], norm[order], dl[order]
    gkey = so_core * (WPC * 2) + so_win * 2 + so_hi
    gstarts = np.searchsorted(gkey, np.arange(NC_ * WPC * 2), side="left")
    rank = np.arange(len(order)) - gstarts[gkey]

    slot_base = (blk_base * 128).astype(np.int64)

    for c in range(NC_):
        m = so_core == c
        e_win, e_hi = so_win[m], so_hi[m]
        e_idx, e_norm, e_dl, e_rank = so_idx16[m], so_norm[m], so_dl[m], rank[m]
        gslot = slot_base[e_win, e_hi] + e_rank

        # padding slots must gather SOME valid row (sel=0 kills the value);
        # spread them across the table — same-row hammering measured ~78%
        # slower on the all-zero index probe (HBM bank serialization)
        min_rows = min(wk * 128 * NC_ for wk in cfg.wchunk)
        idx_flat = ((np.arange(TOTE) * 997) % min_rows).astype(np.int16)
        if pad_idx != 0:
            idx_flat[:] = pad_idx
        idx_flat[gslot] = e_idx
        TOT16 = TOTE // 16
        idxw = np.zeros((128, TOT16), np.int16)
        pos = np.arange(TOTE)
        idxw[pos % 16, pos // 16] = idx_flat
        for r in range(1, 8):
            idxw[r * 16:(r + 1) * 16] = idxw[:16]

        sel = np.zeros((128, TOTE), np.float32)
        jb = gslot >> 7
        pl = gslot & 127
        sel[pl, jb * 128 + e_dl] = e_norm
        # fp8e4m3 (TRN max +-240; norms are <=1): ~3% quantization on edge
        # weights -> ~3.6e-3 final rel err (validated vs f32 reference)
        sel = np.clip(sel, -240, 240).astype(F8NP)

        diag = np.zeros((128, WPC * 128), np.float32)
        q = np.arange(NPC_)
        diag[q & 127, (q >> 7) * 128 + (q & 127)] = selfw[c * NPC_:(c + 1) * NPC_]
        diag = diag.astype(BF)

        xp = np.zeros((PADN, KPAD), np.float32)
        xp[:NPC_, :cfg.d_in] = x[c * NPC_:(c + 1) * NPC_]
        # x streams in fp8e4m3: quantization washes out through the deep
        # averaging stack (simulated 3.6e-3 with fp8 sel; same as bf16 x)
        XH = np.clip(np.ascontiguousarray(
            xp.reshape(WPC, 128, KC, 128).transpose(3, 0, 2, 1).reshape(128, WPC * KC * 128)
        ), -240, 240).astype(F8NP)

        in_maps.append({
            "xh": XH, "w1h": W1H,
            "w2": W234[0], "w3": W234[1], "w4": W234[2],
            "biases": BIAS, "ones": ONES,
            "idxw": idxw, "sel": sel, "diag": diag,
            "tdummy": np.zeros((128, 4), np.float32),
        })

    return in_maps, meta


def build(cfg, meta, with_ag=True, n_layers=4,
          no_gather=False, no_pc_mm=False,
          gather_q4=True, bare_gather=False, dma_scratch=32768):
    """Build the Bass program (shared across all cores).

    with_ag=False: the gathered tables become ExternalInputs (timing variant).
    no_gather / no_pc_mm / bare_gather: timing-only ablations (wrong results).
    """
    BL, BH, sws, blk_base, TOTBLK = (
        meta["BL"], meta["BH"], meta["sws"], meta["blk_base"], meta["TOTBLK"])
    TOTE = TOTBLK * 128
    TOT16 = TOTE // 16
    NC_, WPC, KC = cfg.cores, cfg.wpc, cfg.kc
    PADN = cfg.padn
    W0 = cfg.w0
    WCH = cfg.wchunk
    assert n_layers == 4

    nsq = 4 if gather_q4 else 1
    nc = bacc.Bacc("TRN2", target_bir_lowering=False, debug=False,
                   num_devices=NC_, num_swdge_queues=nsq,
                   dynamic_dma_scratch_size=dma_scratch)

    xh_d = nc.dram_tensor("xh", [128, WPC * KC * 128], F8, kind="ExternalInput")
    w1h_d = nc.dram_tensor("w1h", [128, KC * DH], BF16, kind="ExternalInput")
    w2_d = nc.dram_tensor("w2", [DH, DH], BF16, kind="ExternalInput")
    w3_d = nc.dram_tensor("w3", [DH, DH], BF16, kind="ExternalInput")
    w4_d = nc.dram_tensor("w4", [DH, DH], BF16, kind="ExternalInput")
    bias_d = nc.dram_tensor("biases", [1, 4 * DH], BF16, kind="ExternalInput")
    ones_d = nc.dram_tensor("ones", [1, 128], BF16, kind="ExternalInput")
    idx_d = nc.dram_tensor("idxw", [128, TOT16], I16, kind="ExternalInput")
    sel_d = nc.dram_tensor("sel", [128, TOTE], F8, kind="ExternalInput")
    diag_d = nc.dram_tensor("diag", [128, WPC * 128], BF16, kind="ExternalInput")
    out_d = nc.dram_tensor("out", [PADN, DH], F32, kind="ExternalOutput")
    dummy_d = nc.dram_tensor("tdummy", [128, 4], F32, kind="ExternalInput")
    dummy_o = nc.dram_tensor("tdummy_out", [128, 4], F32, kind="ExternalOutput")

    # per-chunk bounce + gathered tables, double-buffered across layers
    hb = [[nc.dram_tensor(f"hb_{k}_{p}", [WCH[k] * 128, DH], BF16)
           for p in range(2)] for k in range(2)]
    if with_ag:
        hf = [[nc.dram_tensor(f"hfull_{k}_{p}", [NC_ * WCH[k] * 128, DH],
                              BF16, addr_space="Shared")
               for p in range(2)] for k in range(2)]
    else:
        hf = [[nc.dram_tensor(f"hfin_{k}_{p}", [NC_ * WCH[k] * 128, DH],
                              BF16, kind="ExternalInput")
               for p in range(2)] for k in range(2)]

    hb_v = [[hb[k][p].ap().rearrange("(w p) f -> p w f", p=128)
             for p in range(2)] for k in range(2)]
    out_v = out_d.ap().rearrange("(w p) f -> p w f", p=128)

    def chunk_of(w):
        return 0 if w < W0 else 1

    def win_in_chunk(w):
        return w if w < W0 else w - W0

    with tile.TileContext(nc) as tc:
        with (
            tc.tile_pool(name="res", bufs=1) as res,
            tc.tile_pool(name="xstream", bufs=3) as xstream,
            tc.tile_pool(name="gp", bufs=2) as gp,
            tc.tile_pool(name="ep", bufs=3) as ep,
            tc.tile_pool(name="psA", bufs=2, space="PSUM") as psA,
            tc.tile_pool(name="psC", bufs=6, space="PSUM") as psC,
        ):
            # ---- resident loads (once) ----
            w1h_t = res.tile([128, KC * DH], BF16, tag="w1h")
            nc.sync.dma_start(w1h_t[:], w1h_d.ap())
            w2_t = res.tile([DH, DH], BF16, tag="w2")
            nc.sync.dma_start(w2_t[:], w2_d.ap())
            w3_t = res.tile([DH, DH], BF16, tag="w3")
            nc.sync.dma_start(w3_t[:], w3_d.ap())
            w4_t = res.tile([DH, DH], BF16, tag="w4")
            nc.sync.dma_start(w4_t[:], w4_d.ap())
            wl_ts = [None, w2_t, w3_t, w4_t]
            bias_t = res.tile([1, 4 * DH], BF16, tag="bias")
            nc.sync.dma_start(bias_t[:], bias_d.ap())
            ones_t = res.tile([1, 128], BF16, tag="ones")
            nc.sync.dma_start(ones_t[:], ones_d.ap())
            idx_t = res.tile([128, TOT16], I16, tag="idx")
            nc.sync.dma_start(idx_t[:], idx_d.ap())
            diag_t = res.tile([128, WPC * 128], BF16, tag="diag")
            nc.sync.dma_start(diag_t[:], diag_d.ap())
            # sel is fp8 and SBUF-resident for all 4 layers (one 11 MB load
            # replaces 4x22 MB of bf16 streaming)
            if not bare_gather:
                sel_t = res.tile([128, TOTE], F8, tag="sel")
                nc.sync.dma_start(sel_t[:], sel_d.ap())

            hown_t = res.tile([128, WPC * 128], BF16, tag="hown")
            xta = res.tile([128, WPC * 128], BF16, tag="xta")
            xtb = res.tile([128, WPC * 128], BF16, tag="xtb")

            maxblk = max(sw["n_lo"] + sw["n_hi"] for sw in sws)
            g_dummy = None
            if no_gather:
                g_dummy = res.tile([128, maxblk, 128], BF16, tag="gdummy")
                nc.vector.memset(g_dummy[:], 0)

            def emit_ag(l, k):
                if with_ag:
                    p = l % 2
                    nc.gpsimd.collective_compute(
                        "AllGather",
                        mybir.AluOpType.bypass,
                        ins=[hb[k][p].ap().opt()],
                        outs=[hf[k][p].ap().opt()],
                        replica_groups=[list(range(NC_))],
                    )

            def sw_bounds(sw):
                total = sw["n_lo"] + sw["n_hi"]
                return [total * i // nsq for i in range(nsq + 1)]

            def emit_gathers(l, sw):
                # Split the sw's [chunk0 | chunk1] block range into nsq
                # equal-descriptor pieces, one per SWDGE queue (a piece that
                # straddles the phase boundary becomes two gather calls on
                # the same queue). Each piece gets its OWN pool tile so the
                # Pool engine can recycle SBUF at piece granularity instead
                # of stalling on a whole super-window's matmul consumers.
                p = l % 2
                bounds = sw_bounds(sw)
                pieces = []
                for qi in range(nsq):
                    ps_, pe_ = bounds[qi], bounds[qi + 1]
                    if ps_ >= pe_:
                        pieces.append(None)
                        continue
                    gt = gp.tile([128, pe_ - ps_, 128], BF16, tag=f"g{qi}")
                    pieces.append((ps_, gt))
                    for (b0, nb, k) in ((0, sw["n_lo"], 0),
                                        (sw["n_lo"], sw["n_hi"], 1)):
                        s = max(b0, ps_)
                        e = min(b0 + nb, pe_)
                        if s >= e:
                            continue
                        c0 = (sw["lo_blk0"] + s) * 8
                        nn = e - s
                        nc.gpsimd.dma_gather(
                            out_ap=gt[:, s - ps_:e - ps_, :],
                            in_ap=hf[k][p].ap(),
                            idxs_ap=idx_t[:, c0:c0 + nn * 8],
                            num_idxs=nn * 128,
                            num_idxs_reg=nn * 128,
                            elem_size=DH,
                            single_packet=False,
                            queue_num=qi,
                        )
                return pieces

            def g_block(pieces, bounds, j):
                """SBUF slice for the sw-relative gathered block j."""
                for qi in range(nsq):
                    if bounds[qi] <= j < bounds[qi + 1]:
                        ps_, gt = pieces[qi]
                        return gt[:, j - ps_, :]
                raise AssertionError(j)

            def phase_a_window(l, w):
                """h_l[w] = x_{l-1}[w] @ W_l -> hown[w] + hb chunk DMA."""
                pA = psA.tile([128, DH], F32, tag="pA")
                if l == 0:
                    xt = xstream.tile([128, KC * 128], F8, tag="xh")
                    nc.sync.dma_start(
                        xt[:], xh_d.ap()[:, w * KC * 128:(w + 1) * KC * 128])
                    for kc in range(KC):
                        nc.tensor.matmul(
                            pA[:],
                            xt[:, kc * 128:(kc + 1) * 128],
                            w1h_t[:, kc * DH:(kc + 1) * DH],
                            start=(kc == 0), stop=(kc == KC - 1),
                        )
                else:
                    xT = [None, xta, xtb, xta][l]
                    nc.tensor.matmul(
                        pA[:],
                        xT[:, w * 128:(w + 1) * 128],
                        wl_ts[l][:],
                        start=True, stop=True,
                    )
                hw_sl = hown_t[:, w * 128:(w + 1) * 128]
                nc.vector.tensor_copy(hw_sl, pA[:])
                k = chunk_of(w)
                nc.sync.dma_start(hb_v[k][l % 2][:, win_in_chunk(w), :], hw_sl)
                if w == W0 - 1:
                    emit_ag(l, 0)
                elif w == WPC - 1:
                    emit_ag(l, 1)

            if bare_gather:
                for l in range(n_layers):
                    for sw in sws:
                        pieces = emit_gathers(l, sw)
                        first = next(pc for pc in pieces if pc)
                        nc.vector.tensor_copy(hown_t[:, :128],
                                              first[1][:, 0, :])
            else:
                # layer-0 phase A (x stream), AGs issued per chunk
                for w in range(WPC):
                    phase_a_window(0, w)

                for l in range(n_layers):
                    xT_next = [xta, xtb, xta, None][l]
                    last = l == n_layers - 1
                    # next-layer phase A is interleaved with a one-window lag
                    # so its matmul never waits on the just-issued ReLU (an
                    # in-order PE bubble otherwise)
                    pending_a = None

                    for sw in sws:
                        bounds = sw_bounds(sw)
                        if no_gather:
                            def gb_at(j):
                                return g_dummy[:, j, :]
                        else:
                            pieces = emit_gathers(l, sw)

                            def gb_at(j, _p=pieces, _b=bounds):
                                return g_block(_p, _b, j)

                        for w in sw["windows"]:
                            pC = psC.tile([128, DH], F32, tag="pC")
                            ops = []
                            lo_off = int(blk_base[w, 0]) - sw["lo_blk0"]
                            for i in range(BL[w]):
                                j = lo_off + i
                                ja = int(blk_base[w, 0]) + i
                                ops.append((sel_t[:, ja * 128:(ja + 1) * 128],
                                            gb_at(j)))
                            hi_off = int(blk_base[w, 1]) - sw["lo_blk0"]
                            for i in range(BH[w]):
                                j = hi_off + i
                                ja = int(blk_base[w, 1]) + i
                                ops.append((sel_t[:, ja * 128:(ja + 1) * 128],
                                            gb_at(j)))
                            dg = diag_t[:, w * 128:(w + 1) * 128]
                            hw_sl = hown_t[:, w * 128:(w + 1) * 128]
                            bsl = bias_t[0:1, l * DH:(l + 1) * DH]
                            if no_pc_mm:
                                ops = []
                            if not last:
                                mms = [(gb, sb) for (sb, gb) in ops]
                                mms.append((hw_sl, dg))
                                mms.append((bsl, ones_t[0:1, :]))
                            else:
                                mms = list(ops)
                                mms.append((dg, hw_sl))
                                mms.append((ones_t[0:1, :], bsl))
                            for i, (lh, rh) in enumerate(mms):
                                nc.tensor.matmul(
                                    pC[:], lh, rh,
                                    start=(i == 0), stop=(i == len(mms) - 1),
                                )

                            if not last:
                                nc.scalar.activation(
                                    xT_next[:, w * 128:(w + 1) * 128], pC[:],
                                    mybir.ActivationFunctionType.Relu,
                                )
                                if pending_a is not None:
                                    phase_a_window(l + 1, pending_a)
                                pending_a = w
                            else:
                                mx = ep.tile([128, 1], F32, tag="mx")
                                nc.vector.tensor_reduce(
                                    mx[:], pC[:], mybir.AxisListType.X,
                                    mybir.AluOpType.max, negate=True)
                                et = ep.tile([128, DH], F32, tag="et")
                                se = ep.tile([128, 1], F32, tag="se")
                                nc.scalar.activation(
                                    et[:], pC[:],
                                    mybir.ActivationFunctionType.Exp,
                                    bias=mx[:], accum_out=se[:])
                                lnt = ep.tile([128, 1], F32, tag="lnt")
                                nc.scalar.activation(
                                    lnt[:], se[:],
                                    mybir.ActivationFunctionType.Ln)
                                ot = ep.tile([128, DH], F32, tag="ot")
                                nc.vector.tensor_scalar(
                                    ot[:], pC[:], mx[:], lnt[:],
                                    mybir.AluOpType.add,
                                    mybir.AluOpType.subtract)
                                nc.sync.dma_start(out_v[:, w, :], ot[:])

                    if pending_a is not None:
                        phase_a_window(l + 1, pending_a)

            dt_ = res.tile([128, 4], F32, tag="dummy")
            nc.sync.dma_start(dt_[:], dummy_d.ap())
            nc.sync.dma_start(dummy_o.ap(), dt_[:])

    nc.compile()
    return nc


def kernel(**inputs):
    cfg = Cfg(cores=C)
    in_maps, meta = preprocess(cfg, **inputs)
    nc = build(cfg, meta)
    res = run_bass_kernel_spmd(nc, in_maps, core_ids=list(range(C)))
    out = np.concatenate(
        [res.results[c]["out"][:cfg.npc] for c in range(C)], axis=0)
    return np.ascontiguousarray(out, np.float32)


if __name__ == "__main__":
    d = np.load("/root/problem/ref_cache.npz")
    inputs = {k: d[k] for k in
              ("x", "edge_index", "edge_attr", "W1", "b1", "W2", "b2",
               "W3", "b3", "W4", "b4")}
    got = kernel(**inputs)
    exp = d["expected"]
    err = np.abs(got - exp)
    print("abs max err:", err.max(), "rel (absmax):", err.max() / np.abs(exp).max())
